# revision 23
# baseline (speedup 1.0000x reference)
"""Ising log-energy kernel for Trainium2 (8 NeuronCores).

Reference computation (B=512 samples, N=4096 spins on a 64x64 grid):
    e[b] = sum_i u[i]*x[b,i] + sum_{i<j} (binary*mask)[i,j]*x[b,i]*x[b,j]

The mask is the nearest-neighbor upper-triangular grid mask: the only
nonzeros of w = binary*mask sit on the +1 and +64 off-diagonals. So

    e[b] = sum_i x[b,i] * (wr[i]*x[b,i+1] + u[i] + wd[i]*x[b,i+64])

with wr/wd the masked diagonals of `binary`. That's O(B*N) work.

Distribution: tensor-parallel over sites. Core c owns sites
[c*512, c*512+512) for all 512 samples; partial energies are summed on
the host. On-device layout is site-major ([site, batch] = [partition,
free]), 4 chunks of 128 sites per core.

Per chunk k, with t0/t1/t64 = x rows shifted 0/+1/+64 (fp16/fp8 - exact
for +-1 spins) and fp32 per-partition weight columns wr/u/wd:

    DVE/Pool: m1 = t1*wr + u    (tensor_scalar, fp32 AP scalars, fp16 out)
    ACT     : b  = t64*wd       (activation Copy with per-partition scale)
    DVE     : s  = m1 + b       (tensor_tensor, chunk-paired)
    DVE     : p  = t0 * s       (tensor_tensor; exact sign flip)
    PE      : acc[1,512] += ones.T @ p_k   (weights folded out of matmul)

Host buffers are pre-chunked so every DMA is a contiguous [128, X] read
(the V2 strided gathers ran at ~45-85 GB/s; contiguous ~250+). t64 and
t1 chunks are interleaved in one fp8 buffer so each quarter-DMA
unblocks both the ACT b-op and the DVE m1-op for that chunk.
"""

import os
from contextlib import ExitStack
import sys

import numpy as np

for _p in ("/opt/trn_rl_repo", "/root/.axon_site/_ro/trn_rl_repo"):
    if os.path.isdir(_p) and _p not in sys.path:
        sys.path.insert(0, _p)

import ml_dtypes

import concourse.bass as bass
import concourse.mybir as mybir
from concourse.bass_utils import run_bass_kernel_spmd


N = 4096          # total spins (64x64 grid)
NG = 64           # grid side (down-neighbor stride)
B = 512           # batch
NCORES = 8
S = N // NCORES   # sites per core = 512
NCHUNK = S // 128  # 128-site chunks per core = 4

FP32 = mybir.dt.float32
FP16 = mybir.dt.float16
FP8 = mybir.dt.float8e4

AOP = mybir.AluOpType
AFT = mybir.ActivationFunctionType

NP_FP16 = np.float16
NP_FP8 = ml_dtypes.float8_e4m3

POOL_M1 = False  # compute m1 for chunks 2,3 on the Pool engine


def _build_bass():
    """Raw Bass (no Tile): the local walrus build only encodes ONE sync
    wait per instruction, so all waits are standalone wait_ge on counting
    semaphores. DMAs of one tensor share one ring (FIFO per ring) so
    cumulative semaphore values are race-free."""
    nc = bass.Bass()
    # bufA quarter k = [t64 chunk k | t1 chunk k], each [128, 512] fp8.
    bufA = nc.declare_dram_parameter("bufA", [128, 2 * NCHUNK * B], FP8, isOutput=False)
    bufT0 = nc.declare_dram_parameter("bufT0", [128, NCHUNK * B], FP16, isOutput=False)
    wts = nc.declare_dram_parameter("wts", [128, 3 * NCHUNK], FP32, isOutput=False)
    out = nc.declare_dram_parameter("out", [1, B], FP16, isOutput=True)

    with ExitStack() as ctx:
        w = ctx.enter_context(nc.sbuf_tensor("w", [128, 3 * NCHUNK], FP32))
        tA = ctx.enter_context(nc.sbuf_tensor("tA", [128, 2 * NCHUNK * B], FP8))
        t0 = ctx.enter_context(nc.sbuf_tensor("t0", [128, NCHUNK * B], FP16))
        m1 = ctx.enter_context(nc.sbuf_tensor("m1", [128, NCHUNK * B], FP16))
        bb = ctx.enter_context(nc.sbuf_tensor("bb", [128, NCHUNK * B], FP16))
        ss = ctx.enter_context(nc.sbuf_tensor("ss", [128, NCHUNK * B], FP16))
        pp = ctx.enter_context(nc.sbuf_tensor("pp", [128, NCHUNK * B], FP16))
        ones = ctx.enter_context(nc.sbuf_tensor("ones", [128, 1], FP16))
        scr = ctx.enter_context(nc.sbuf_tensor("scr", [1, 1], FP16))
        res = ctx.enter_context(nc.sbuf_tensor("res", [1, B], FP16))
        acc = ctx.enter_context(nc.psum_tensor("acc", [1, B], FP32))
        # One semaphore per waited DMA: the 16 SDMA engines' sub-increments
        # interleave across concurrent DMAs, so intermediate cumulative
        # values of a shared semaphore are NOT race-free.
        sA = [
            ctx.enter_context(nc.semaphore(f"sA{q}")) for q in range(NCHUNK)
        ]
        st0 = [ctx.enter_context(nc.semaphore(f"st0{h}")) for h in range(2)]
        sq = ctx.enter_context(nc.semaphore("sq"))
        swt = ctx.enter_context(nc.semaphore("swt"))
        sb = ctx.enter_context(nc.semaphore("sb"))
        sm = ctx.enter_context(nc.semaphore("sm"))
        sv = ctx.enter_context(nc.semaphore("sv"))
        sp = ctx.enter_context(nc.semaphore("sp"))
        sr = ctx.enter_context(nc.semaphore("sr"))
        so = ctx.enter_context(nc.semaphore("so"))
        block = ctx.enter_context(nc.Block())

        def t64c(k):
            return tA[:, 2 * k * B : (2 * k + 1) * B]

        def t1c(k):
            return tA[:, (2 * k + 1) * B : (2 * k + 2) * B]

        def ch(tt, k, n=1):
            return tt[:, k * B : (k + n) * B]

        def wcol(k, j):
            return w[:, 3 * k + j : 3 * k + j + 1]

        # sync ring (FIFO): ALL x-traffic in priority order - the four
        # [t64|t1] quarters (gate DVE+ACT chunk k), then t0 halves
        # (needed later, by p), then out-store. One ring avoids the
        # packet-granularity round-robin between rings that delayed the
        # critical first quarter in V3.
        @block.sync
        def _(sync):
            sync.dma_start(out=tA[:, :B], in_=bufA[:, :B]).then_inc(sq, 16)
            sync.dma_start(
                out=tA[:, B : 2 * B], in_=bufA[:, B : 2 * B]
            ).then_inc(sA[0], 16)
            for q in range(1, NCHUNK):
                sync.dma_start(
                    out=tA[:, q * 2 * B : (q + 1) * 2 * B],
                    in_=bufA[:, q * 2 * B : (q + 1) * 2 * B],
                ).then_inc(sA[q], 16)

        # ACT: dummy activation first (forces the one-time ACT_TABLE_LOAD
        # while DMAs fly), then the b ops; no DMA-issue work here.
        @block.scalar
        def _(scalar):
            scalar.activation(scr[:], ones[0:1, 0:1], AFT.Copy, scale=1.0)
            scalar.wait_ge(swt, 16)
            for k in range(NCHUNK):
                scalar.activation(
                    ch(bb, k), t64c(k), AFT.Copy, scale=wcol(k, 2)
                )._wait_ge(sq if k == 0 else sA[k], 16).then_inc(sb, 1)
            scalar.activation(res[:], acc[:], AFT.Copy)._wait_ge(sp, 1)
            scalar.dma_start(out=out[:], in_=res[:]).then_inc(so, 16)

        # pool: weights (tiny, SWDGE) + m1 for chunks 2,3.
        @block.gpsimd
        def _(gpsimd):
            gpsimd.dma_start(out=w[:], in_=wts[:]).then_inc(swt, 16)
            for h in range(2):
                gpsimd.dma_start(
                    out=t0[:, h * 2 * B : (h + 1) * 2 * B],
                    in_=bufT0[:, h * 2 * B : (h + 1) * 2 * B],
                ).then_inc(st0[h], 16)
            if POOL_M1:
                gpsimd.wait_ge(swt, 16)
                for k in (2, 3):
                    gpsimd.wait_ge(sA[k], 16)
                    gpsimd.tensor_scalar(
                        ch(m1, k), t1c(k), wcol(k, 0), wcol(k, 1),
                        AOP.mult, AOP.add,
                    ).then_inc(sm, 1)

        @block.vector
        def _(vector):
            vector.memset(ones[:], 1.0)
            vector.wait_ge(swt, 16)
            for k in range(NCHUNK):
                vector.wait_ge(sA[k], 16)
                vector.tensor_scalar(
                    ch(m1, k), t1c(k), wcol(k, 0), wcol(k, 1), AOP.mult, AOP.add
                )
            vector.wait_ge(sb, 2)
            vector.tensor_add(ch(ss, 0, 2), ch(m1, 0, 2), ch(bb, 0, 2))
            vector.wait_ge(st0[0], 16)
            vector.tensor_mul(ch(pp, 0, 2), ch(t0, 0, 2), ch(ss, 0, 2)).then_inc(
                sv, 1
            )
            vector.wait_ge(sb, 4)
            if POOL_M1:
                vector.wait_ge(sm, 2)
            vector.tensor_add(ch(ss, 2, 2), ch(m1, 2, 2), ch(bb, 2, 2))
            vector.wait_ge(st0[1], 16)
            # split the tail so the last PE matmul starts one op earlier
            vector.tensor_mul(ch(pp, 2), ch(t0, 2), ch(ss, 2))._wait_ge(
                st0[1], 16
            ).then_inc(sv, 1)
            vector.tensor_mul(ch(pp, 3), ch(t0, 3), ch(ss, 3)).then_inc(sv, 1)

        @block.tensor
        def _(tensor):
            tensor.matmul(acc[:], ones[:], ch(pp, 0), start=True, stop=False)._wait_ge(
                sv, 1
            )
            tensor.matmul(acc[:], ones[:], ch(pp, 1), start=False, stop=False)
            tensor.matmul(acc[:], ones[:], ch(pp, 2), start=False, stop=False)._wait_ge(
                sv, 2
            )
            tensor.matmul(acc[:], ones[:], ch(pp, 3), start=False, stop=True)._wait_ge(
                sv, 3
            ).then_inc(sp, 1)

    return nc


_NC_CACHE = None


def _get_nc():
    global _NC_CACHE
    if _NC_CACHE is None:
        _NC_CACHE = _build_bass()
    return _NC_CACHE


def _prep_inputs(x, unary, binary, mask):
    """Host-side shard prep: masked diagonals + pre-chunked spin tiles."""
    wr = np.zeros(N, np.float32)
    wd = np.zeros(N, np.float32)
    wr[: N - 1] = np.diagonal(binary, 1) * np.diagonal(mask, 1)
    wd[: N - NG] = np.diagonal(binary, NG) * np.diagonal(mask, NG)
    u = np.asarray(unary, np.float32)

    PADROWS = N + NG + 1
    xt = np.zeros((PADROWS, B), np.float32)
    xt[:N] = np.asarray(x, np.float32).T
    xt16 = xt.astype(NP_FP16)
    xt8 = xt.astype(NP_FP8)

    in_maps = []
    for c in range(NCORES):
        base = c * S
        w = np.empty((128, 3 * NCHUNK), np.float32)
        bufA = np.empty((128, 2 * NCHUNK, B), NP_FP8)
        bufT0 = np.empty((128, NCHUNK, B), NP_FP16)
        for k in range(NCHUNK):
            r0 = base + k * 128
            w[:, 3 * k + 0] = wr[r0 : r0 + 128]
            w[:, 3 * k + 1] = u[r0 : r0 + 128]
            w[:, 3 * k + 2] = wd[r0 : r0 + 128]
            bufA[:, 2 * k] = xt8[r0 + NG : r0 + NG + 128]    # t64 chunk k
            bufA[:, 2 * k + 1] = xt16[r0 + 1 : r0 + 1 + 128].astype(NP_FP8)
            bufT0[:, k] = xt16[r0 : r0 + 128]                # t0 chunk k
        in_maps.append(
            {
                "bufA": bufA.reshape(128, 2 * NCHUNK * B),
                "bufT0": bufT0.reshape(128, NCHUNK * B),
                "wts": w,
            }
        )
    return in_maps


def kernel(x, unary, binary, mask):
    nc = _get_nc()
    in_maps = _prep_inputs(x, unary, binary, mask)
    res = run_bass_kernel_spmd(nc, in_maps, list(range(NCORES))).results
    parts = np.stack([np.asarray(r["out"], np.float32) for r in res])  # [8,1,B]
    return parts.sum(axis=(0, 1), dtype=np.float64).astype(np.float32)


# revision 24
# speedup vs baseline: 1.0225x; 1.0225x over previous
"""Ising log-energy kernel for Trainium2 (8 NeuronCores).

Reference computation (B=512 samples, N=4096 spins on a 64x64 grid):
    e[b] = sum_i u[i]*x[b,i] + sum_{i<j} (binary*mask)[i,j]*x[b,i]*x[b,j]

The mask is the nearest-neighbor upper-triangular grid mask: the only
nonzeros of w = binary*mask sit on the +1 and +64 off-diagonals. So

    e[b] = sum_i x[b,i] * (wr[i]*x[b,i+1] + u[i] + wd[i]*x[b,i+64])

with wr/wd the masked diagonals of `binary`. That's O(B*N) work.

Distribution: tensor-parallel over sites. Core c owns sites
[c*512, c*512+512) for all 512 samples; partial energies are summed on
the host. On-device layout is site-major ([site, batch] = [partition,
free]), 4 chunks of 128 sites per core.

Per chunk k, with t0/t1/t64 = x rows shifted 0/+1/+64 (fp16/fp8 - exact
for +-1 spins) and fp32 per-partition weight columns wr/u/wd:

    DVE/Pool: m1 = t1*wr + u    (tensor_scalar, fp32 AP scalars, fp16 out)
    ACT     : b  = t64*wd       (activation Copy with per-partition scale)
    DVE     : s  = m1 + b       (tensor_tensor, chunk-paired)
    DVE     : p  = t0 * s       (tensor_tensor; exact sign flip)
    PE      : acc[1,512] += ones.T @ p_k   (weights folded out of matmul)

Host buffers are pre-chunked so every DMA is a contiguous [128, X] read
(the V2 strided gathers ran at ~45-85 GB/s; contiguous ~250+). t64 and
t1 chunks are interleaved in one fp8 buffer so each quarter-DMA
unblocks both the ACT b-op and the DVE m1-op for that chunk.
"""

import os
from contextlib import ExitStack
import sys

import numpy as np

for _p in ("/opt/trn_rl_repo", "/root/.axon_site/_ro/trn_rl_repo"):
    if os.path.isdir(_p) and _p not in sys.path:
        sys.path.insert(0, _p)

import ml_dtypes

import concourse.bass as bass
import concourse.mybir as mybir
from concourse.bass_utils import run_bass_kernel_spmd


N = 4096          # total spins (64x64 grid)
NG = 64           # grid side (down-neighbor stride)
B = 512           # batch
NCORES = 8
S = N // NCORES   # sites per core = 512
NCHUNK = S // 128  # 128-site chunks per core = 4

FP32 = mybir.dt.float32
FP16 = mybir.dt.float16
FP8 = mybir.dt.float8e4

AOP = mybir.AluOpType
AFT = mybir.ActivationFunctionType

NP_FP16 = np.float16
NP_FP8 = ml_dtypes.float8_e4m3

POOL_M1 = False  # compute m1 for chunks 2,3 on the Pool engine


def _build_bass():
    """Raw Bass (no Tile): the local walrus build only encodes ONE sync
    wait per instruction, so all waits are standalone wait_ge on counting
    semaphores. DMAs of one tensor share one ring (FIFO per ring) so
    cumulative semaphore values are race-free."""
    nc = bass.Bass()
    # bufA quarter k = [t64 chunk k | t1 chunk k], each [128, 512] fp8.
    bufA = nc.declare_dram_parameter("bufA", [128, 2 * NCHUNK * B], FP8, isOutput=False)
    bufT0 = nc.declare_dram_parameter("bufT0", [128, NCHUNK * B], FP16, isOutput=False)
    wts = nc.declare_dram_parameter("wts", [128, 3 * NCHUNK], FP32, isOutput=False)
    out = nc.declare_dram_parameter("out", [1, B], FP16, isOutput=True)

    with ExitStack() as ctx:
        w = ctx.enter_context(nc.sbuf_tensor("w", [128, 3 * NCHUNK], FP32))
        tA = ctx.enter_context(nc.sbuf_tensor("tA", [128, 2 * NCHUNK * B], FP8))
        t0 = ctx.enter_context(nc.sbuf_tensor("t0", [128, NCHUNK * B], FP16))
        m1 = ctx.enter_context(nc.sbuf_tensor("m1", [128, NCHUNK * B], FP16))
        bb = ctx.enter_context(nc.sbuf_tensor("bb", [128, NCHUNK * B], FP16))
        ss = ctx.enter_context(nc.sbuf_tensor("ss", [128, NCHUNK * B], FP16))
        pp = ctx.enter_context(nc.sbuf_tensor("pp", [128, NCHUNK * B], FP16))
        ones = ctx.enter_context(nc.sbuf_tensor("ones", [128, 1], FP16))
        scr = ctx.enter_context(nc.sbuf_tensor("scr", [1, 1], FP16))
        res = ctx.enter_context(nc.sbuf_tensor("res", [1, B], FP16))
        acc = ctx.enter_context(nc.psum_tensor("acc", [1, B], FP32))
        # One semaphore per waited DMA: the 16 SDMA engines' sub-increments
        # interleave across concurrent DMAs, so intermediate cumulative
        # values of a shared semaphore are NOT race-free.
        sA = [
            ctx.enter_context(nc.semaphore(f"sA{q}")) for q in range(NCHUNK)
        ]
        st0 = [ctx.enter_context(nc.semaphore(f"st0{h}")) for h in range(2)]
        sq = ctx.enter_context(nc.semaphore("sq"))
        swt = ctx.enter_context(nc.semaphore("swt"))
        sb = ctx.enter_context(nc.semaphore("sb"))
        sm = ctx.enter_context(nc.semaphore("sm"))
        sv = ctx.enter_context(nc.semaphore("sv"))
        sp = ctx.enter_context(nc.semaphore("sp"))
        sr = ctx.enter_context(nc.semaphore("sr"))
        so = ctx.enter_context(nc.semaphore("so"))
        block = ctx.enter_context(nc.Block())

        def t64c(k):
            return tA[:, 2 * k * B : (2 * k + 1) * B]

        def t1c(k):
            return tA[:, (2 * k + 1) * B : (2 * k + 2) * B]

        def ch(tt, k, n=1):
            return tt[:, k * B : (k + n) * B]

        def wcol(k, j):
            return w[:, 3 * k + j : 3 * k + j + 1]

        # sync ring (FIFO): ALL x-traffic in priority order - the four
        # [t64|t1] quarters (gate DVE+ACT chunk k), then t0 halves
        # (needed later, by p), then out-store. One ring avoids the
        # packet-granularity round-robin between rings that delayed the
        # critical first quarter in V3.
        @block.sync
        def _(sync):
            sync.dma_start(out=tA[:, :B], in_=bufA[:, :B]).then_inc(sq, 16)
            sync.dma_start(
                out=tA[:, B : 2 * B], in_=bufA[:, B : 2 * B]
            ).then_inc(sA[0], 16)
            for q in range(1, NCHUNK):
                sync.dma_start(
                    out=tA[:, q * 2 * B : (q + 1) * 2 * B],
                    in_=bufA[:, q * 2 * B : (q + 1) * 2 * B],
                ).then_inc(sA[q], 16)

        # ACT: dummy activation first (forces the one-time ACT_TABLE_LOAD
        # while DMAs fly), then the b ops; no DMA-issue work here.
        @block.scalar
        def _(scalar):
            scalar.activation(scr[:], ones[0:1, 0:1], AFT.Copy, scale=1.0)
            scalar.wait_ge(swt, 16)
            for k in range(NCHUNK):
                scalar.activation(
                    ch(bb, k), t64c(k), AFT.Copy, scale=wcol(k, 2)
                )._wait_ge(sq if k == 0 else sA[k], 16).then_inc(sb, 1)
            scalar.activation(res[:], acc[:], AFT.Copy)._wait_ge(sp, 1)
            scalar.dma_start(out=out[:], in_=res[:]).then_inc(so, 16)

        # pool: weights (tiny, SWDGE) + m1 for chunks 2,3.
        @block.gpsimd
        def _(gpsimd):
            gpsimd.dma_start(out=w[:], in_=wts[:]).then_inc(swt, 16)
            for h in range(2):
                gpsimd.dma_start(
                    out=t0[:, h * 2 * B : (h + 1) * 2 * B],
                    in_=bufT0[:, h * 2 * B : (h + 1) * 2 * B],
                ).then_inc(st0[h], 16)
            if POOL_M1:
                gpsimd.wait_ge(swt, 16)
                for k in (2, 3):
                    gpsimd.wait_ge(sA[k], 16)
                    gpsimd.tensor_scalar(
                        ch(m1, k), t1c(k), wcol(k, 0), wcol(k, 1),
                        AOP.mult, AOP.add,
                    ).then_inc(sm, 1)

        @block.vector
        def _(vector):
            vector.memset(ones[:], 1.0)
            vector.wait_ge(swt, 16)
            for k in range(NCHUNK):
                vector.tensor_scalar(
                    ch(m1, k), t1c(k), wcol(k, 0), wcol(k, 1), AOP.mult, AOP.add
                )._wait_ge(sA[k], 16)
            vector.tensor_add(
                ch(ss, 0, 2), ch(m1, 0, 2), ch(bb, 0, 2)
            )._wait_ge(sb, 2)
            vector.tensor_mul(
                ch(pp, 0, 2), ch(t0, 0, 2), ch(ss, 0, 2)
            )._wait_ge(st0[0], 16).then_inc(sv, 1)
            vector.tensor_add(
                ch(ss, 2, 2), ch(m1, 2, 2), ch(bb, 2, 2)
            )._wait_ge(sb, 4)
            # split the tail so the last PE matmul starts one op earlier
            vector.tensor_mul(ch(pp, 2), ch(t0, 2), ch(ss, 2))._wait_ge(
                st0[1], 16
            ).then_inc(sv, 1)
            vector.tensor_mul(ch(pp, 3), ch(t0, 3), ch(ss, 3)).then_inc(sv, 1)

        @block.tensor
        def _(tensor):
            tensor.matmul(acc[:], ones[:], ch(pp, 0), start=True, stop=False)._wait_ge(
                sv, 1
            )
            tensor.matmul(acc[:], ones[:], ch(pp, 1), start=False, stop=False)
            tensor.matmul(acc[:], ones[:], ch(pp, 2), start=False, stop=False)._wait_ge(
                sv, 2
            )
            tensor.matmul(acc[:], ones[:], ch(pp, 3), start=False, stop=True)._wait_ge(
                sv, 3
            ).then_inc(sp, 1)

    return nc


_NC_CACHE = None


def _get_nc():
    global _NC_CACHE
    if _NC_CACHE is None:
        _NC_CACHE = _build_bass()
    return _NC_CACHE


def _prep_inputs(x, unary, binary, mask):
    """Host-side shard prep: masked diagonals + pre-chunked spin tiles."""
    wr = np.zeros(N, np.float32)
    wd = np.zeros(N, np.float32)
    wr[: N - 1] = np.diagonal(binary, 1) * np.diagonal(mask, 1)
    wd[: N - NG] = np.diagonal(binary, NG) * np.diagonal(mask, NG)
    u = np.asarray(unary, np.float32)

    PADROWS = N + NG + 1
    xt = np.zeros((PADROWS, B), np.float32)
    xt[:N] = np.asarray(x, np.float32).T
    xt16 = xt.astype(NP_FP16)
    xt8 = xt.astype(NP_FP8)

    in_maps = []
    for c in range(NCORES):
        base = c * S
        w = np.empty((128, 3 * NCHUNK), np.float32)
        bufA = np.empty((128, 2 * NCHUNK, B), NP_FP8)
        bufT0 = np.empty((128, NCHUNK, B), NP_FP16)
        for k in range(NCHUNK):
            r0 = base + k * 128
            w[:, 3 * k + 0] = wr[r0 : r0 + 128]
            w[:, 3 * k + 1] = u[r0 : r0 + 128]
            w[:, 3 * k + 2] = wd[r0 : r0 + 128]
            bufA[:, 2 * k] = xt8[r0 + NG : r0 + NG + 128]    # t64 chunk k
            bufA[:, 2 * k + 1] = xt16[r0 + 1 : r0 + 1 + 128].astype(NP_FP8)
            bufT0[:, k] = xt16[r0 : r0 + 128]                # t0 chunk k
        in_maps.append(
            {
                "bufA": bufA.reshape(128, 2 * NCHUNK * B),
                "bufT0": bufT0.reshape(128, NCHUNK * B),
                "wts": w,
            }
        )
    return in_maps


def kernel(x, unary, binary, mask):
    nc = _get_nc()
    in_maps = _prep_inputs(x, unary, binary, mask)
    res = run_bass_kernel_spmd(nc, in_maps, list(range(NCORES))).results
    parts = np.stack([np.asarray(r["out"], np.float32) for r in res])  # [8,1,B]
    return parts.sum(axis=(0, 1), dtype=np.float64).astype(np.float32)
